# revision 1
# baseline (speedup 1.0000x reference)
"""Trainium2 Bass kernel for an attention layer.

Computes, for each batch element b:
    q      = x @ W                  [T, D]
    scores = q @ x^T                [T, T]
    out    = softmax(scores) @ x    [T, D]

with B=8, T=4096, D=64, f32 in/out. Sharding: data-parallel over batch,
one batch element per NeuronCore (8 cores), W replicated. No collectives.

Per-core algorithm (flash-style, scores never touch HBM):
  - xT [D, T] via PE transposes, bf16
  - qT [D, T] = W^T stationary matmul over xT, bf16
  - x_aug [T, D+1] row-blocks with a ones column (row sums come free)
  - per 512-col panel of t: for each 128-row block of s:
      scoresT block = xT_blk^T @ qT_panel -> PSUM f32
      exp (ScalarE, no max subtraction; scores bounded ~22) -> SBUF bf16
      accumulate o_augT[65, 512] += x_aug_blk^T @ expT_blk (PSUM f32)
    epilogue: PE-transpose to [128, 65], divide by the sums column, DMA out.
"""

import numpy as np

B, T, D = 8, 4096, 64
P = 128                 # SBUF/PSUM partitions
NBLK = T // P           # 32 row blocks of s'
PW = 512                # panel width (t columns per panel)
NPANEL = T // PW        # 8 panels
DA = D + 1              # augmented with ones column

# Pack pairs of K=64 score matmuls into the two 64-row halves of the PE
# array (tile_position row tiling) — ~2x score-matmul throughput.
import os as _os
ROW_TILED = _os.environ.get("KERNEL_ROW_TILED", "0") == "1"


def build_bass(stage="full"):
    import concourse.bacc as bacc
    import concourse.mybir as mybir
    import concourse.tile as tile
    from concourse.masks import make_identity

    f32 = mybir.dt.float32
    bf16 = mybir.dt.bfloat16
    EXP = mybir.ActivationFunctionType.Exp

    nc = bacc.Bacc("TRN2", target_bir_lowering=False, debug=False, num_devices=B)

    x_ext = nc.dram_tensor("x", [T, D], f32, kind="ExternalInput")
    w_ext = nc.dram_tensor("W", [D, D], f32, kind="ExternalInput")
    out_ext = nc.dram_tensor("out", [T, D], f32, kind="ExternalOutput")

    # x viewed as [p, blk, d]: row t = blk*128 + p
    x_view = x_ext.ap().rearrange("(b p) d -> p b d", p=P)
    out_view = out_ext.ap().rearrange("(b p) d -> b p d", p=P)

    with tile.TileContext(nc) as tc:
        with (
            tc.tile_pool(name="const", bufs=1) as const,
            tc.tile_pool(name="sb", bufs=1) as sb,
            tc.tile_pool(name="aux_ps", bufs=2, space="PSUM") as aux_ps,
            tc.tile_pool(name="sc_ps", bufs=2, space="PSUM") as sc_ps,
            tc.tile_pool(name="o_ps", bufs=2, space="PSUM") as o_ps,
            tc.tile_pool(name="exps", bufs=3) as exps,
            tc.tile_pool(name="osb", bufs=2) as osb,
            tc.tile_pool(name="small", bufs=4) as small,
        ):
            ident = const.tile([P, P], f32)
            make_identity(nc, ident[:])

            x_sb = sb.tile([P, NBLK, D], f32)       # x rows on partitions
            nc.sync.dma_start(out=x_sb[:], in_=x_view)

            w_sb = const.tile([D, D], f32)
            nc.sync.dma_start(out=w_sb[:], in_=w_ext.ap())
            w_bf = const.tile([D, D], bf16)
            nc.vector.tensor_copy(w_bf[:], w_sb[:])

            # x_aug: [P, NBLK, DA] bf16 with ones in the last column
            x_aug = sb.tile([P, NBLK, DA], bf16)
            nc.vector.memset(x_aug[:], 1.0)
            nc.vector.tensor_copy(x_aug[:, :, 0:D], x_sb[:])

            # xT [D, T] bf16 via PE transposes; when row-tiling, duplicate
            # onto both partition halves so score matmuls can run 2
            # concurrent K=64 matmuls in the 128-row PE array
            xparts = P if ROW_TILED else D
            xT = sb.tile([xparts, T], bf16)
            for r in range(NBLK // 4):
                tp = aux_ps.tile([D, 4 * P], f32, tag="aux")
                for j in range(4):
                    blk = 4 * r + j
                    nc.tensor.transpose(
                        tp[:, j * P:(j + 1) * P], x_sb[:, blk, :], ident[:]
                    )
                sl = slice(r * 4 * P, (r + 1) * 4 * P)
                nc.vector.tensor_copy(xT[0:D, sl], tp[:])
                if ROW_TILED:
                    nc.vector.tensor_copy(xT[D:2 * D, sl], tp[:])

            # qT [D, T] bf16 = W^T @ xT (stationary W, K = D)
            qT = sb.tile([xparts, T], bf16)
            for j in range(NPANEL):
                qp = aux_ps.tile([D, PW], f32, tag="aux")
                nc.tensor.matmul(
                    qp[:], w_bf[:], xT[0:D, j * PW:(j + 1) * PW],
                    start=True, stop=True,
                )
                sl = slice(j * PW, (j + 1) * PW)
                nc.vector.tensor_copy(qT[0:D, sl], qp[:])
                if ROW_TILED:
                    nc.vector.tensor_copy(qT[D:2 * D, sl], qp[:])

            if stage == "prologue":
                # debug: dump qT into out rows (reinterpret out as [64, 4096])
                out_dbg = out_ext.ap().rearrange("(a b) d -> a (b d)", a=D)
                nc.gpsimd.dma_start(out=out_dbg, in_=qT[0:D, :])
            # main loop
            panels = [] if stage == "prologue" else (
                [0] if stage == "panel1" else list(range(NPANEL)))
            # 32 row blocks per panel in pairs: each exp instruction
            # covers a 2-bank PSUM tile
            GRP = [2] * 16
            for pnl in panels:
                op = o_ps.tile([DA, PW], f32)
                k0 = 0
                for g in GRP:
                    sc = sc_ps.tile([P, 2 * PW], f32, tag="sc")
                    for h in range(g):
                        k = k0 + h
                        base = D * (k % 2) if ROW_TILED else 0
                        nc.tensor.matmul(
                            sc[:, h * PW:(h + 1) * PW],
                            xT[base:base + D, k * P:(k + 1) * P],
                            qT[base:base + D, pnl * PW:(pnl + 1) * PW],
                            start=True, stop=True,
                        )
                    ex = exps.tile([P, 2 * PW], bf16, tag="ex")
                    nc.scalar.activation(
                        ex[:, 0:g * PW], sc[:, 0:g * PW], EXP
                    )
                    for h in range(g):
                        k = k0 + h
                        nc.tensor.matmul(
                            op[:],
                            x_aug[:, k, :],
                            ex[:, h * PW:(h + 1) * PW],
                            start=(k == 0), stop=(k == NBLK - 1),
                        )
                    k0 += g
                # epilogue: transpose, normalize, store
                ob = osb.tile([DA, PW], f32)
                nc.vector.tensor_copy(ob[:], op[:])
                for j in range(PW // P):
                    tp2 = aux_ps.tile([P, DA], f32, tag="aux")
                    nc.tensor.transpose(
                        tp2[:], ob[:, j * P:(j + 1) * P], ident[0:DA, 0:DA]
                    )
                    rc = small.tile([P, 1], f32, tag="rc")
                    nc.vector.reciprocal(rc[:], tp2[:, D:DA])
                    rs = small.tile([P, D], f32, tag="rs")
                    nc.vector.tensor_scalar_mul(rs[:], tp2[:, 0:D], rc[:])
                    nc.sync.dma_start(
                        out=out_view[pnl * (PW // P) + j], in_=rs[:]
                    )

    if not nc.is_finalized():
        nc.finalize()
    return nc


def kernel(inputs: np.ndarray, W: np.ndarray) -> np.ndarray:
    from concourse.bass_utils import run_bass_kernel_spmd

    nc = build_bass()
    x = np.ascontiguousarray(np.asarray(inputs, dtype=np.float32))
    w = np.ascontiguousarray(np.asarray(W, dtype=np.float32))
    in_maps = [{"x": x[i], "W": w} for i in range(B)]
    res = run_bass_kernel_spmd(nc, in_maps, core_ids=list(range(B)))
    out = np.stack([res.results[i]["out"] for i in range(B)], axis=0)
    return out.astype(np.float32)


if __name__ == "__main__":
    rng = np.random.default_rng(0)
    x = rng.standard_normal((B, T, D), dtype=np.float32)
    w = (rng.standard_normal((D, D)) * 0.05).astype(np.float32)
    out = kernel(inputs=x, W=w)
    print("out", out.shape, out.dtype)



# revision 5
# speedup vs baseline: 1.0373x; 1.0373x over previous
"""Trainium2 Bass kernel for an attention layer.

Computes, for each batch element b:
    q      = x @ W                  [T, D]
    scores = q @ x^T                [T, T]
    out    = softmax(scores) @ x    [T, D]

with B=8, T=4096, D=64, f32 in/out. Sharding: data-parallel over batch,
one batch element per NeuronCore (8 cores), W replicated. No collectives.

Per-core algorithm (flash-style, scores never touch HBM):
  - x rows mapped to SBUF partition p = t // 32 (contiguous 8KB DMA
    descriptors per partition; the induced row permutation is applied
    identically to the s axis, the t axis and the output, so it cancels).
  - xT [128, T] bf16 via PE transposes; partitions 64-127 duplicate
    0-63 (SBUF->SBUF DMA) so score matmuls run as 2 concurrent K=64
    matmuls (PE row tiling).
  - qT [128, T] bf16 = W^T-stationary matmul over xT, same duplication.
  - x_aug [128, blk, 65] bf16 with a ones column (row sums come free).
  - per 512-col panel of t, per pair of 128-row s-blocks:
      scoresT pair -> PSUM f32 [128, 1024]
      exp -> SBUF bf16: ScalarE (table exp) for most pairs, VectorE for
      the rest via a fused Schraudolph: i16 = round(s*128*log2e +
      128*(127-C)) is exactly the bit pattern of bf16 2^(s*log2e - C),
      one tensor_scalar instruction per pair.
      o_augT[65, 512] += x_aug_blk^T @ expT_blk (PSUM f32, accumulated)
    accum matmuls are emitted one pair behind the score matmuls so the
    in-order PE queue never head-blocks on the exp engines.
    epilogue: PE-transpose to [128, 65], divide by the sums column,
    stage into an SBUF accumulator; one contiguous output DMA at the end.
"""

import numpy as np

B, T, D = 8, 4096, 64
P = 128                 # SBUF/PSUM partitions
NBLK = T // P           # 32 row blocks of s
PW = 512                # panel width (t columns per panel)
NPANEL = T // PW        # 8 panels
DA = D + 1              # augmented with ones column
NPAIR = NBLK // 2       # 16 block pairs per panel

# Schraudolph constants: i16 = round(s * SCHRAUD_A + SCHRAUD_B) viewed as
# bf16 is 2^(s*log2e - C) ~= exp(s). C trades max error for mean error.
LOG2E = 1.4426950408889634
SCHRAUD_C = 0.0570
SCHRAUD_A = 128.0 * LOG2E
SCHRAUD_B = 128.0 * (127.0 - SCHRAUD_C)

# Which of the 16 pairs per panel the DVE handles (rest go to ScalarE).
DVE_PAIRS = frozenset({1, 4, 7, 9, 11, 13})


def build_bass(stage="full", dve_pairs=DVE_PAIRS):
    import concourse.bacc as bacc
    import concourse.mybir as mybir
    import concourse.tile as tile
    from concourse.masks import make_identity

    f32 = mybir.dt.float32
    bf16 = mybir.dt.bfloat16
    i16 = mybir.dt.int16
    EXP = mybir.ActivationFunctionType.Exp
    MULT = mybir.AluOpType.mult
    ADD = mybir.AluOpType.add

    nc = bacc.Bacc("TRN2", target_bir_lowering=False, debug=False, num_devices=B)

    x_ext = nc.dram_tensor("x", [T, D], f32, kind="ExternalInput")
    w_ext = nc.dram_tensor("W", [D, D], f32, kind="ExternalInput")
    out_ext = nc.dram_tensor("out", [T, D], f32, kind="ExternalOutput")

    # row t = p*NBLK + j: partition p's rows are contiguous in DRAM, so the
    # in/out DMAs are 128 descriptors of 8KB instead of 4096 of 256B.
    x_view = x_ext.ap().rearrange("(p j) d -> p j d", p=P)
    out_view = out_ext.ap().rearrange("(p j) d -> p (j d)", p=P)

    with tile.TileContext(nc) as tc:
        with (
            tc.tile_pool(name="const", bufs=1) as const,
            tc.tile_pool(name="sb", bufs=1) as sb,
            tc.tile_pool(name="sc_ps", bufs=3, space="PSUM") as sc_ps,
            tc.tile_pool(name="o_ps", bufs=1, space="PSUM") as o_ps,
            tc.tile_pool(name="tp2_ps", bufs=1, space="PSUM") as tp2_ps,
            tc.tile_pool(name="exps", bufs=4) as exps,
            tc.tile_pool(name="small", bufs=4) as small,
        ):
            ident = const.tile([P, P], f32)
            make_identity(nc, ident[:])

            x_sb = sb.tile([P, NBLK, D], f32)       # x rows on partitions
            nc.sync.dma_start(out=x_sb[:], in_=x_view)

            w_sb = const.tile([D, D], f32)
            nc.sync.dma_start(out=w_sb[:], in_=w_ext.ap())
            w_bf = const.tile([D, D], bf16)
            nc.vector.tensor_copy(w_bf[:], w_sb[:])

            # x_aug: [P, NBLK, DA] bf16 with ones in the last column
            x_aug = sb.tile([P, NBLK, DA], bf16)
            nc.vector.memset(x_aug[:, :, D:DA], 1.0)
            half = NBLK // 2
            nc.vector.tensor_copy(x_aug[:, 0:half, 0:D], x_sb[:, 0:half, :])
            nc.scalar.copy(x_aug[:, half:NBLK, 0:D], x_sb[:, half:NBLK, :])

            # xT [128, T] bf16 via PE transposes (rows 0-63), then one
            # SBUF->SBUF DMA duplicates onto rows 64-127 for row tiling.
            xT = sb.tile([P, T], bf16)
            for r in range(NBLK // 4):
                tp = sc_ps.tile([P, 2 * PW], f32, tag="sc")
                for j in range(4):
                    blk = 4 * r + j
                    nc.tensor.transpose(
                        tp[0:D, j * P:(j + 1) * P], x_sb[:, blk, :],
                        ident[:],
                    )
                sl = slice(r * 4 * P, (r + 1) * 4 * P)
                if r % 2 == 0:
                    nc.vector.tensor_copy(xT[0:D, sl], tp[0:D, 0:4 * P])
                else:
                    nc.scalar.copy(xT[0:D, sl], tp[0:D, 0:4 * P])
            nc.sync.dma_start(out=xT[D:2 * D, :], in_=xT[0:D, :])

            # qT [128, T] bf16 = W^T @ xT (stationary W, K = D)
            qT = sb.tile([P, T], bf16)
            for j in range(NPANEL):
                qp = sc_ps.tile([P, 2 * PW], f32, tag="sc")
                nc.tensor.matmul(
                    qp[0:D, 0:PW], w_bf[:], xT[0:D, j * PW:(j + 1) * PW],
                    start=True, stop=True,
                )
                sl = slice(j * PW, (j + 1) * PW)
                if j % 2 == 0:
                    nc.vector.tensor_copy(qT[0:D, sl], qp[0:D, 0:PW])
                else:
                    nc.scalar.copy(qT[0:D, sl], qp[0:D, 0:PW])
            nc.sync.dma_start(out=qT[D:2 * D, :], in_=qT[0:D, :])

            if stage == "prologue":
                out_dbg = out_ext.ap().rearrange("(a b) d -> a (b d)", a=D)
                nc.gpsimd.dma_start(out=out_dbg, in_=qT[0:D, :])

            osb_all = sb.tile([P, NBLK, D], f32)    # staged output rows

            panels = [] if stage == "prologue" else (
                [0] if stage == "panel1" else list(range(NPANEL)))

            def emit_scores(pnl, g):
                """Score matmuls + exp for pair g of panel pnl."""
                sc = sc_ps.tile([P, 2 * PW], f32, tag="sc")
                for h in range(2):
                    k = 2 * g + h
                    base = D * (k % 2)
                    nc.tensor.matmul(
                        sc[:, h * PW:(h + 1) * PW],
                        xT[base:base + D, k * P:(k + 1) * P],
                        qT[base:base + D, pnl * PW:(pnl + 1) * PW],
                        start=True, stop=True,
                    )
                ex = exps.tile([P, 2 * PW], bf16, tag="ex")
                if g in dve_pairs:
                    nc.vector.tensor_scalar(
                        out=ex[:].bitcast(i16), in0=sc[:],
                        scalar1=float(SCHRAUD_A), scalar2=float(SCHRAUD_B),
                        op0=MULT, op1=ADD,
                    )
                else:
                    nc.scalar.activation(ex[:], sc[:], EXP)
                return ex

            def emit_accum(g, ex, op):
                for h in range(2):
                    k = 2 * g + h
                    nc.tensor.matmul(
                        op[:],
                        x_aug[:, k, :],
                        ex[:, h * PW:(h + 1) * PW],
                        start=(k == 0), stop=(k == NBLK - 1),
                    )

            def emit_epilogue(pnl, ob):
                """Transpose + normalize + stage panel pnl's output."""
                tp2 = tp2_ps.tile([P, 4, DA], f32, tag="tp2")
                for j2 in range(4):
                    nc.tensor.transpose(
                        tp2[:, j2, :], ob[:, j2 * P:(j2 + 1) * P],
                        ident[0:DA, 0:DA],
                    )
                for j2 in range(4):
                    jj = pnl * 4 + j2
                    rc = small.tile([P, 1], f32, tag="rc")
                    nc.vector.reciprocal(rc[:], tp2[:, j2, D:DA])
                    nc.vector.tensor_scalar(
                        out=osb_all[:, jj, :], in0=tp2[:, j2, 0:D],
                        scalar1=rc[:], scalar2=None, op0=MULT,
                    )

            prev = None  # (pnl, ob) pending epilogue
            for pnl in panels:
                op = o_ps.tile([DA, PW], f32, tag="o")
                exq = []
                for g in range(NPAIR):
                    exq.append(emit_scores(pnl, g))
                    if g == 1 and prev is not None:
                        emit_epilogue(*prev)
                        prev = None
                    if g >= 1:
                        emit_accum(g - 1, exq[g - 1], op)
                emit_accum(NPAIR - 1, exq[NPAIR - 1], op)
                ob = small.tile([DA, PW], f32, tag="ob")
                nc.scalar.copy(ob[:], op[:])
                prev = (pnl, ob)
            if prev is not None:
                emit_epilogue(*prev)

            if stage != "prologue":
                nc.sync.dma_start(
                    out=out_view,
                    in_=osb_all[:].rearrange("p j d -> p (j d)"),
                )

    if not nc.is_finalized():
        nc.finalize()
    return nc


def kernel(inputs: np.ndarray, W: np.ndarray) -> np.ndarray:
    from concourse.bass_utils import run_bass_kernel_spmd

    nc = build_bass()
    x = np.ascontiguousarray(np.asarray(inputs, dtype=np.float32))
    w = np.ascontiguousarray(np.asarray(W, dtype=np.float32))
    in_maps = [{"x": x[i], "W": w} for i in range(B)]
    res = run_bass_kernel_spmd(nc, in_maps, core_ids=list(range(B)))
    out = np.stack([res.results[i]["out"] for i in range(B)], axis=0)
    return out.astype(np.float32)


if __name__ == "__main__":
    rng = np.random.default_rng(0)
    x = rng.standard_normal((B, T, D), dtype=np.float32)
    w = (rng.standard_normal((D, D)) * 0.05).astype(np.float32)
    out = kernel(inputs=x, W=w)
    print("out", out.shape, out.dtype)


# revision 11
# speedup vs baseline: 20.4974x; 19.7595x over previous
"""Trainium2 Bass kernel for an attention layer.

Computes, for each batch element b:
    q      = x @ W                  [T, D]
    scores = q @ x^T                [T, T]
    out    = softmax(scores) @ x    [T, D]

with B=8, T=4096, D=64, f32 in/out. Sharding: data-parallel over batch,
one batch element per NeuronCore (8 cores), W replicated. No collectives.

Per-core algorithm (flash-style, scores never touch HBM):
  - x rows mapped to SBUF partition p = t // 32 (contiguous 8KB DMA
    descriptors per partition; the induced row permutation is applied
    identically to the s axis, the t axis and the output, so it cancels).
  - xT [128, T] bf16 via PE transposes; partitions 64-127 duplicate
    0-63 (SBUF->SBUF DMA) so score matmuls run as 2 concurrent K=64
    matmuls (PE row tiling).
  - qT [128, T] bf16 = W^T-stationary matmul over xT, same duplication.
  - x_aug [128, blk, 65] bf16 with a ones column (row sums come free).
  - per 512-col panel of t, per pair of 128-row s-blocks:
      scoresT pair -> PSUM f32 [128, 1024]
      exp -> SBUF bf16: ScalarE (table exp) for most pairs, VectorE for
      the rest via a fused Schraudolph: i16 = round(s*128*log2e +
      128*(127-C)) is exactly the bit pattern of bf16 2^(s*log2e - C),
      one tensor_scalar instruction per pair.
      o_augT[65, 512] += x_aug_blk^T @ expT_blk (PSUM f32, accumulated)
    accum matmuls are emitted one pair behind the score matmuls so the
    in-order PE queue never head-blocks on the exp engines.
    epilogue: PE-transpose to [128, 65], divide by the sums column,
    stage into an SBUF accumulator; one contiguous output DMA at the end.
"""

import numpy as np

B, T, D = 8, 4096, 64
P = 128                 # SBUF/PSUM partitions
NBLK = T // P           # 32 row blocks of s
PW = 512                # panel width (t columns per panel)
NPANEL = T // PW        # 8 panels
DA = D + 1              # augmented with ones column
NPAIR = NBLK // 2       # 16 block pairs per panel

# Schraudolph constants: i16 = round(s * SCHRAUD_A + SCHRAUD_B) viewed as
# bf16 is 2^(s*log2e - C) ~= exp(s). C trades max error for mean error.
LOG2E = 1.4426950408889634
SCHRAUD_C = 0.0570
SCHRAUD_A = 128.0 * LOG2E
SCHRAUD_B = 128.0 * (127.0 - SCHRAUD_C)

# Which of the 16 pairs per panel the DVE handles (rest go to ScalarE).
DVE_PAIRS = frozenset({1, 4, 7, 9, 11, 13})


def build_bass(stage="full", dve_pairs=DVE_PAIRS, repeat=1):
    import concourse.bacc as bacc
    import concourse.mybir as mybir
    import concourse.tile as tile
    from concourse.masks import make_identity

    f32 = mybir.dt.float32
    bf16 = mybir.dt.bfloat16
    i16 = mybir.dt.int16
    EXP = mybir.ActivationFunctionType.Exp
    MULT = mybir.AluOpType.mult
    ADD = mybir.AluOpType.add

    nc = bacc.Bacc("TRN2", target_bir_lowering=False, debug=False, num_devices=B)

    x_ext = nc.dram_tensor("x", [T, D], f32, kind="ExternalInput")
    w_ext = nc.dram_tensor("W", [D, D], f32, kind="ExternalInput")
    out_ext = nc.dram_tensor("out", [T, D], f32, kind="ExternalOutput")

    # row t = p*NBLK + j: partition p's rows are contiguous in DRAM, so the
    # in/out DMAs are 128 descriptors of 8KB instead of 4096 of 256B.
    x_view = x_ext.ap().rearrange("(p j) d -> p j d", p=P)
    out_view = out_ext.ap().rearrange("(p j) d -> p (j d)", p=P)

    with tile.TileContext(nc) as tc:
        with (
            tc.tile_pool(name="const", bufs=1) as const,
            tc.tile_pool(name="sb", bufs=1) as sb,
            tc.tile_pool(name="sc_ps", bufs=3, space="PSUM") as sc_ps,
            tc.tile_pool(name="o_ps", bufs=1, space="PSUM") as o_ps,
            tc.tile_pool(name="tp2_ps", bufs=1, space="PSUM") as tp2_ps,
            tc.tile_pool(name="exps", bufs=4) as exps,
            tc.tile_pool(name="small", bufs=4) as small,
        ):
            ident = const.tile([P, P], f32)
            make_identity(nc, ident[:])

            x_sb = sb.tile([P, NBLK, D], f32)       # x rows on partitions
            w_sb = const.tile([D, D], f32)
            w_bf = const.tile([D, D], bf16)
            x_aug = sb.tile([P, NBLK, DA], bf16)
            xT = sb.tile([P, T], bf16)
            qT = sb.tile([P, T], bf16)
            osb_all = sb.tile([P, NBLK, D], f32)    # staged output rows

            panels = [] if stage == "prologue" else (
                [0] if stage == "panel1" else list(range(NPANEL)))

            def emit_scores(pnl, g):
                """Score matmuls + exp for pair g of panel pnl."""
                sc = sc_ps.tile([P, 2 * PW], f32, tag="sc")
                for h in range(2):
                    k = 2 * g + h
                    base = D * (k % 2)
                    nc.tensor.matmul(
                        sc[:, h * PW:(h + 1) * PW],
                        xT[base:base + D, k * P:(k + 1) * P],
                        qT[base:base + D, pnl * PW:(pnl + 1) * PW],
                        start=True, stop=True,
                    )
                ex = exps.tile([P, 2 * PW], bf16, tag="ex")
                if g in dve_pairs:
                    nc.vector.tensor_scalar(
                        out=ex[:].bitcast(i16), in0=sc[:],
                        scalar1=float(SCHRAUD_A), scalar2=float(SCHRAUD_B),
                        op0=MULT, op1=ADD,
                    )
                else:
                    nc.scalar.activation(ex[:], sc[:], EXP)
                return ex

            def emit_accum(g, ex, op):
                for h in range(2):
                    k = 2 * g + h
                    nc.tensor.matmul(
                        op[:],
                        x_aug[:, k, :],
                        ex[:, h * PW:(h + 1) * PW],
                        start=(k == 0), stop=(k == NBLK - 1),
                    )

            def emit_epilogue(pnl, ob):
                """Transpose + normalize + stage panel pnl's output."""
                tp2 = tp2_ps.tile([P, 4, DA], f32, tag="tp2")
                for j2 in range(4):
                    nc.tensor.transpose(
                        tp2[:, j2, :], ob[:, j2 * P:(j2 + 1) * P],
                        ident[0:DA, 0:DA],
                    )
                for j2 in range(4):
                    jj = pnl * 4 + j2
                    rc = small.tile([P, 1], f32, tag="rc")
                    nc.vector.reciprocal(rc[:], tp2[:, j2, D:DA])
                    nc.vector.tensor_scalar(
                        out=osb_all[:, jj, :], in0=tp2[:, j2, 0:D],
                        scalar1=rc[:], scalar2=None, op0=MULT,
                    )

            for rep in range(repeat):
                # x on the SP HWDGE queue (2 chunks so transposes can start
                # early), W on the ACT queue so the loads run in parallel.
                nc.sync.dma_start(out=x_sb[:, 0:NBLK // 2, :],
                                  in_=x_view[:, 0:NBLK // 2, :])
                nc.sync.dma_start(out=x_sb[:, NBLK // 2:NBLK, :],
                                  in_=x_view[:, NBLK // 2:NBLK, :])
                nc.scalar.dma_start(out=w_sb[:], in_=w_ext.ap())
                nc.vector.tensor_copy(w_bf[:], w_sb[:])

                # x_aug: [P, NBLK, DA] bf16 with ones in the last column
                nc.vector.memset(x_aug[:, :, D:DA], 1.0)
                half = NBLK // 2
                nc.vector.tensor_copy(x_aug[:, 0:half, 0:D], x_sb[:, 0:half, :])
                nc.scalar.copy(x_aug[:, half:NBLK, 0:D], x_sb[:, half:NBLK, :])

                # xT [128, T] bf16 via PE transposes (rows 0-63), then one
                # SBUF->SBUF DMA duplicates onto rows 64-127 for row tiling.
                for r in range(NBLK // 4):
                    tp = sc_ps.tile([P, 2 * PW], f32, tag="sc")
                    for j in range(4):
                        blk = 4 * r + j
                        nc.tensor.transpose(
                            tp[0:D, j * P:(j + 1) * P], x_sb[:, blk, :],
                            ident[:],
                        )
                    sl = slice(r * 4 * P, (r + 1) * 4 * P)
                    if r % 2 == 0:
                        nc.vector.tensor_copy(xT[0:D, sl], tp[0:D, 0:4 * P])
                    else:
                        nc.scalar.copy(xT[0:D, sl], tp[0:D, 0:4 * P])
                    nc.sync.dma_start(out=xT[D:2 * D, sl], in_=xT[0:D, sl])

                # qT [128, T] bf16 = W^T @ xT (stationary W, K = D)
                for j in range(NPANEL):
                    qp = sc_ps.tile([P, 2 * PW], f32, tag="sc")
                    nc.tensor.matmul(
                        qp[0:D, 0:PW], w_bf[:], xT[0:D, j * PW:(j + 1) * PW],
                        start=True, stop=True,
                    )
                    sl = slice(j * PW, (j + 1) * PW)
                    if j % 2 == 0:
                        nc.vector.tensor_copy(qT[0:D, sl], qp[0:D, 0:PW])
                    else:
                        nc.scalar.copy(qT[0:D, sl], qp[0:D, 0:PW])
                    nc.sync.dma_start(out=qT[D:2 * D, sl], in_=qT[0:D, sl])

                if stage == "prologue":
                    out_dbg = out_ext.ap().rearrange("(a b) d -> a (b d)", a=D)
                    nc.gpsimd.dma_start(out=out_dbg, in_=qT[0:D, :])

                prev = None  # (pnl, ob) pending epilogue
                for pnl in panels:
                    op = o_ps.tile([DA, PW], f32, tag="o")
                    exq = []
                    for g in range(NPAIR):
                        exq.append(emit_scores(pnl, g))
                        if g == 1 and prev is not None:
                            emit_epilogue(*prev)
                            prev = None
                        if g >= 1:
                            emit_accum(g - 1, exq[g - 1], op)
                    emit_accum(NPAIR - 1, exq[NPAIR - 1], op)
                    ob = small.tile([DA, PW], f32, tag="ob")
                    nc.scalar.copy(ob[:], op[:])
                    prev = (pnl, ob)
                if prev is not None:
                    emit_epilogue(*prev)

                if stage != "prologue":
                    nc.sync.dma_start(
                        out=out_view,
                        in_=osb_all[:].rearrange("p j d -> p (j d)"),
                    )

    if not nc.is_finalized():
        nc.finalize()
    return nc


def kernel(inputs: np.ndarray, W: np.ndarray) -> np.ndarray:
    from concourse.bass_utils import run_bass_kernel_spmd

    nc = build_bass()
    x = np.ascontiguousarray(np.asarray(inputs, dtype=np.float32))
    w = np.ascontiguousarray(np.asarray(W, dtype=np.float32))
    in_maps = [{"x": x[i], "W": w} for i in range(B)]
    res = run_bass_kernel_spmd(nc, in_maps, core_ids=list(range(B)))
    out = np.stack([res.results[i]["out"] for i in range(B)], axis=0)
    return out.astype(np.float32)


if __name__ == "__main__":
    rng = np.random.default_rng(0)
    x = rng.standard_normal((B, T, D), dtype=np.float32)
    w = (rng.standard_normal((D, D)) * 0.05).astype(np.float32)
    out = kernel(inputs=x, W=w)
    print("out", out.shape, out.dtype)
